# revision 11
# baseline (speedup 1.0000x reference)
"""Derivative1D kernel for Trainium2 (8 NeuronCores, data-parallel over batch).

Reference: y = x[:, 1:, :] - x[:, :-1, :] with x of shape (64, 16384, 32) f32.

Key observation: flattening each batch's (L, C) block to a contiguous array,
y_flat[i] = x_flat[i + C] - x_flat[i].  The row the reference drops (l = L-1)
absorbs the batch-boundary garbage, so the whole per-core problem is one flat
shifted subtraction; the garbage rows are sliced off on the host.

Sharding: batch axis across 8 cores (8 batches per core, no communication).

Per core: chunks of the flat input are loaded with *overlapping* rows
(partition p reads F+C elements starting at p*F) so the shift-by-C stays
inside each partition; one vector subtract per chunk; contiguous store.

Raw Bass (no TileContext): the walrus codegen on this path rejects
instructions carrying more than one sync wait, so every wait is an explicit
standalone wait_ge.  Per-slot semaphores are used because DMA completions
are not FIFO across queues — each wait certifies exactly the DMA it needs.
"""

import numpy as np

B, L, C = 64, 16384, 32
NCORES = 8
BLOC = B // NCORES            # batches per core
N = BLOC * L * C              # flat elements per core
PAD = C                       # shift amount = channel count
P = 128                       # SBUF partitions
F = 8192                      # free elements per partition per chunk
CHUNK = P * F                 # elements per chunk
NCHUNKS = N // CHUNK
NBUF = 2                      # in-tile slots
NOBUF = 2                     # out-tile slots

assert N % CHUNK == 0

_built = None


def build_bass():
    global _built
    if _built is not None:
        return _built
    import concourse.bass as bass
    import concourse.mybir as mybir
    from contextlib import ExitStack

    f32 = mybir.dt.float32
    nc = bass.Bass()
    x = nc.declare_dram_parameter("x", [N + PAD], f32, isOutput=False)
    y = nc.declare_dram_parameter("y", [N], f32, isOutput=True)

    with ExitStack() as ctx:
        A = [
            ctx.enter_context(nc.sbuf_tensor(f"A{i}", [P, F + PAD], f32))
            for i in range(NBUF)
        ]
        Y = [
            ctx.enter_context(nc.sbuf_tensor(f"Y{i}", [P, F], f32))
            for i in range(NOBUF)
        ]
        LS = [ctx.enter_context(nc.semaphore(f"LS{i}")) for i in range(NBUF)]
        SS = [ctx.enter_context(nc.semaphore(f"SS{i}")) for i in range(NOBUF)]
        VS = ctx.enter_context(nc.semaphore("VS"))

        block = ctx.enter_context(nc.Block())

        @block.sync
        def _(sync):
            # Single HWDGE ring: the ring drains near-serially, so reads
            # and writes alternate at whole-DMA granularity (4 MiB) —
            # coarse enough that HBM R/W turnaround is negligible.  (A
            # dual-ring split was measured worse: both rings interleave
            # at packet granularity on every SDMA engine.)
            for k in range(min(NBUF, NCHUNKS)):
                sync.dma_start(
                    out=A[k][:],
                    in_=bass.AP(x, k * CHUNK, [[F, P], [1, F + PAD]]),
                ).then_inc(LS[k], 16)
            for k in range(NCHUNKS):
                # Store chunk k once its subtract is done; the wait also
                # certifies (in program order) that the next load's input
                # slot has been fully read by its subtract.
                sync.wait_ge(VS, k + 1)
                sync.dma_start(
                    out=bass.AP(y, k * CHUNK, [[F, P], [1, F]]),
                    in_=Y[k % NOBUF][:],
                ).then_inc(SS[k % NOBUF], 16)
                kn = k + NBUF
                if kn < NCHUNKS:
                    sync.dma_start(
                        out=A[kn % NBUF][:],
                        in_=bass.AP(x, kn * CHUNK, [[F, P], [1, F + PAD]]),
                    ).then_inc(LS[kn % NBUF], 16)
            # Epilogue: all stores complete before the kernel exits.
            for i in range(NOBUF):
                n_stores = len(range(i, NCHUNKS, NOBUF))
                sync.wait_ge(SS[i], 16 * n_stores)

        @block.vector
        def _(vector):
            for k in range(NCHUNKS):
                vector.wait_ge(LS[k % NBUF], 16 * (k // NBUF + 1))
                if k >= NOBUF:
                    # WAR: the out slot must have been drained by its store.
                    vector.wait_ge(SS[k % NOBUF], 16 * ((k - NOBUF) // NOBUF + 1))
                a = A[k % NBUF]
                vector.tensor_sub(
                    Y[k % NOBUF][:], a[:, PAD : F + PAD], a[:, 0:F]
                ).then_inc(VS, 1)

    _built = nc
    return nc


def _shard_inputs(x: np.ndarray) -> list[dict]:
    in_maps = []
    for c in range(NCORES):
        shard = np.empty(N + PAD, dtype=np.float32)
        shard[:N] = x[c * BLOC : (c + 1) * BLOC].reshape(-1)
        shard[N:] = 0.0
        in_maps.append({"x": shard})
    return in_maps


def _gather_outputs(results: list[dict]) -> np.ndarray:
    y = np.empty((B, L - 1, C), dtype=np.float32)
    for c in range(NCORES):
        y[c * BLOC : (c + 1) * BLOC] = (
            results[c]["y"].reshape(BLOC, L, C)[:, : L - 1, :]
        )
    return y


def kernel(x: np.ndarray) -> np.ndarray:
    from concourse.bass_utils import run_bass_kernel_spmd

    nc = build_bass()
    x = np.asarray(x, dtype=np.float32)
    res = run_bass_kernel_spmd(nc, _shard_inputs(x), list(range(NCORES)))
    return _gather_outputs(res.results)


# revision 12
# speedup vs baseline: 1.1253x; 1.1253x over previous
"""Derivative1D kernel for Trainium2 (8 NeuronCores, data-parallel over batch).

Reference: y = x[:, 1:, :] - x[:, :-1, :] with x of shape (64, 16384, 32) f32.

Key observation: flattening each batch's (L, C) block to a contiguous array,
y_flat[i] = x_flat[i + C] - x_flat[i].  The row the reference drops (l = L-1)
absorbs the batch-boundary garbage, so the whole per-core problem is one flat
shifted subtraction; the garbage rows are sliced off on the host.

Sharding: batch axis across 8 cores (8 batches per core, no communication).

Per core: chunks of the flat input are loaded with *overlapping* rows
(partition p reads F+C elements starting at p*F) so the shift-by-C stays
inside each partition; one vector subtract per chunk; contiguous store.

Raw Bass (no TileContext): the walrus codegen on this path rejects
instructions carrying more than one sync wait, so every wait is an explicit
standalone wait_ge.  Per-slot semaphores are used because DMA completions
are not FIFO across queues — each wait certifies exactly the DMA it needs.
"""

import numpy as np

B, L, C = 64, 16384, 32
NCORES = 8
BLOC = B // NCORES            # batches per core
N = BLOC * L * C              # flat elements per core
PAD = C                       # shift amount = channel count
P = 128                       # SBUF partitions
F = 8192                      # free elements per partition per chunk
CHUNK = P * F                 # elements per chunk
NCHUNKS = N // CHUNK
NBUF = 2                      # in-tile slots
NOBUF = 2                     # out-tile slots

assert N % CHUNK == 0

_built = None


def build_bass():
    global _built
    if _built is not None:
        return _built
    import concourse.bass as bass
    import concourse.mybir as mybir
    from contextlib import ExitStack

    f32 = mybir.dt.float32
    nc = bass.Bass()
    x = nc.declare_dram_parameter("x", [N + PAD], f32, isOutput=False)
    y = nc.declare_dram_parameter("y", [N], f32, isOutput=True)

    with ExitStack() as ctx:
        A = [
            ctx.enter_context(nc.sbuf_tensor(f"A{i}", [P, F + PAD], f32))
            for i in range(NBUF)
        ]
        Y = [
            ctx.enter_context(nc.sbuf_tensor(f"Y{i}", [P, F], f32))
            for i in range(NOBUF)
        ]
        LS = [ctx.enter_context(nc.semaphore(f"LS{i}")) for i in range(NBUF)]
        SS = [ctx.enter_context(nc.semaphore(f"SS{i}")) for i in range(NOBUF)]
        VS = ctx.enter_context(nc.semaphore("VS"))

        # no_gpsimd_drain: skip the expensive GpSimd dge_drain + full
        # EVSEM butterfly at block exit (no SWDGE DMAs are in flight;
        # HWDGE completion is certified by the explicit SS waits).
        block = ctx.enter_context(nc.Block(no_gpsimd_drain=True))

        @block.sync
        def _(sync):
            # Single HWDGE ring: the ring drains near-serially, so reads
            # and writes alternate at whole-DMA granularity (4 MiB) —
            # coarse enough that HBM R/W turnaround is negligible.  (A
            # dual-ring split was measured worse: both rings interleave
            # at packet granularity on every SDMA engine.)
            for k in range(min(NBUF, NCHUNKS)):
                sync.dma_start(
                    out=A[k][:],
                    in_=bass.AP(x, k * CHUNK, [[F, P], [1, F + PAD]]),
                ).then_inc(LS[k], 16)
            for k in range(NCHUNKS):
                # Store chunk k once its subtract is done; the wait also
                # certifies (in program order) that the next load's input
                # slot has been fully read by its subtract.
                sync.wait_ge(VS, k + 1)
                sync.dma_start(
                    out=bass.AP(y, k * CHUNK, [[F, P], [1, F]]),
                    in_=Y[k % NOBUF][:],
                ).then_inc(SS[k % NOBUF], 16)
                kn = k + NBUF
                if kn < NCHUNKS:
                    sync.dma_start(
                        out=A[kn % NBUF][:],
                        in_=bass.AP(x, kn * CHUNK, [[F, P], [1, F + PAD]]),
                    ).then_inc(LS[kn % NBUF], 16)
            # Epilogue: all stores complete before the kernel exits.
            for i in range(NOBUF):
                n_stores = len(range(i, NCHUNKS, NOBUF))
                sync.wait_ge(SS[i], 16 * n_stores)

        @block.vector
        def _(vector):
            for k in range(NCHUNKS):
                vector.wait_ge(LS[k % NBUF], 16 * (k // NBUF + 1))
                if k >= NOBUF:
                    # WAR: the out slot must have been drained by its store.
                    vector.wait_ge(SS[k % NOBUF], 16 * ((k - NOBUF) // NOBUF + 1))
                a = A[k % NBUF]
                vector.tensor_sub(
                    Y[k % NOBUF][:], a[:, PAD : F + PAD], a[:, 0:F]
                ).then_inc(VS, 1)

    _built = nc
    return nc


def _shard_inputs(x: np.ndarray) -> list[dict]:
    in_maps = []
    for c in range(NCORES):
        shard = np.empty(N + PAD, dtype=np.float32)
        shard[:N] = x[c * BLOC : (c + 1) * BLOC].reshape(-1)
        shard[N:] = 0.0
        in_maps.append({"x": shard})
    return in_maps


def _gather_outputs(results: list[dict]) -> np.ndarray:
    y = np.empty((B, L - 1, C), dtype=np.float32)
    for c in range(NCORES):
        y[c * BLOC : (c + 1) * BLOC] = (
            results[c]["y"].reshape(BLOC, L, C)[:, : L - 1, :]
        )
    return y


def kernel(x: np.ndarray) -> np.ndarray:
    from concourse.bass_utils import run_bass_kernel_spmd

    nc = build_bass()
    x = np.asarray(x, dtype=np.float32)
    res = run_bass_kernel_spmd(nc, _shard_inputs(x), list(range(NCORES)))
    return _gather_outputs(res.results)


# revision 18
# speedup vs baseline: 1.1416x; 1.0145x over previous
"""Derivative1D kernel for Trainium2 (8 NeuronCores, data-parallel over batch).

Reference: y = x[:, 1:, :] - x[:, :-1, :] with x of shape (64, 16384, 32) f32.

Key observation: flattening each batch's (L, C) block to a contiguous array,
y_flat[i] = x_flat[i + C] - x_flat[i].  The row the reference drops (l = L-1)
absorbs the batch-boundary garbage, so the whole per-core problem is one flat
shifted subtraction; the garbage rows are sliced off on the host.

Sharding: batch axis across 8 cores (8 batches per core, no communication).

Per core: chunks of the flat input are loaded with *overlapping* rows
(partition p reads F+C elements starting at p*F) so the shift-by-C stays
inside each partition; one vector subtract per chunk; contiguous store.

Raw Bass (no TileContext): the walrus codegen on this path rejects
instructions carrying more than one sync wait, so every wait is an explicit
standalone wait_ge.  Per-slot semaphores are used because DMA completions
are not FIFO across queues — each wait certifies exactly the DMA it needs.
"""

import numpy as np

B, L, C = 64, 16384, 32
NCORES = 8
BLOC = B // NCORES            # batches per core
N = BLOC * L * C              # flat elements per core
PAD = C                       # shift amount = channel count
P = 128                       # SBUF partitions
F = 8192                      # free elements per partition per chunk
CHUNK = P * F                 # elements per chunk
NCHUNKS = N // CHUNK
NBUF = 3                      # in-tile slots
NOBUF = 3                     # out-tile slots

assert N % CHUNK == 0

_built = None


def build_bass():
    global _built
    if _built is not None:
        return _built
    import concourse.bass as bass
    import concourse.mybir as mybir
    from contextlib import ExitStack

    f32 = mybir.dt.float32
    nc = bass.Bass()
    x = nc.declare_dram_parameter("x", [N + PAD], f32, isOutput=False)
    y = nc.declare_dram_parameter("y", [N], f32, isOutput=True)

    with ExitStack() as ctx:
        A = [
            ctx.enter_context(nc.sbuf_tensor(f"A{i}", [P, F + PAD], f32))
            for i in range(NBUF)
        ]
        Y = [
            ctx.enter_context(nc.sbuf_tensor(f"Y{i}", [P, F], f32))
            for i in range(NOBUF)
        ]
        LS = [ctx.enter_context(nc.semaphore(f"LS{i}")) for i in range(NBUF)]
        SS = [ctx.enter_context(nc.semaphore(f"SS{i}")) for i in range(NOBUF)]
        VS = ctx.enter_context(nc.semaphore("VS"))

        # no_gpsimd_drain: skip the expensive GpSimd dge_drain + full
        # EVSEM butterfly at block exit (no SWDGE DMAs are in flight;
        # HWDGE completion is certified by the explicit SS waits).
        block = ctx.enter_context(nc.Block(no_gpsimd_drain=True))

        @block.sync
        def _(sync):
            # Single HWDGE ring: the ring drains near-serially, so reads
            # and writes alternate at whole-DMA granularity (4 MiB) —
            # coarse enough that HBM R/W turnaround is negligible.  (A
            # dual-ring split was measured worse: both rings interleave
            # at packet granularity on every SDMA engine.)
            for k in range(min(NBUF, NCHUNKS)):
                sync.dma_start(
                    out=A[k][:],
                    in_=bass.AP(x, k * CHUNK, [[F, P], [1, F + PAD]]),
                ).then_inc(LS[k], 16)
            for k in range(NCHUNKS):
                # Store chunk k once its subtract is done; the wait also
                # certifies (in program order) that the next load's input
                # slot has been fully read by its subtract.
                sync.wait_ge(VS, k + 1)
                sync.dma_start(
                    out=bass.AP(y, k * CHUNK, [[F, P], [1, F]]),
                    in_=Y[k % NOBUF][:],
                ).then_inc(SS[k % NOBUF], 16)
                kn = k + NBUF
                if kn < NCHUNKS:
                    sync.dma_start(
                        out=A[kn % NBUF][:],
                        in_=bass.AP(x, kn * CHUNK, [[F, P], [1, F + PAD]]),
                    ).then_inc(LS[kn % NBUF], 16)
            # Epilogue: all stores complete before the kernel exits.
            for i in range(NOBUF):
                n_stores = len(range(i, NCHUNKS, NOBUF))
                sync.wait_ge(SS[i], 16 * n_stores)

        @block.vector
        def _(vector):
            for k in range(NCHUNKS):
                vector.wait_ge(LS[k % NBUF], 16 * (k // NBUF + 1))
                if k >= NOBUF:
                    # WAR: the out slot must have been drained by its store.
                    vector.wait_ge(SS[k % NOBUF], 16 * ((k - NOBUF) // NOBUF + 1))
                a = A[k % NBUF]
                vector.tensor_sub(
                    Y[k % NOBUF][:], a[:, PAD : F + PAD], a[:, 0:F]
                ).then_inc(VS, 1)

    _built = nc
    return nc


def _shard_inputs(x: np.ndarray) -> list[dict]:
    in_maps = []
    for c in range(NCORES):
        shard = np.empty(N + PAD, dtype=np.float32)
        shard[:N] = x[c * BLOC : (c + 1) * BLOC].reshape(-1)
        shard[N:] = 0.0
        in_maps.append({"x": shard})
    return in_maps


def _gather_outputs(results: list[dict]) -> np.ndarray:
    y = np.empty((B, L - 1, C), dtype=np.float32)
    for c in range(NCORES):
        y[c * BLOC : (c + 1) * BLOC] = (
            results[c]["y"].reshape(BLOC, L, C)[:, : L - 1, :]
        )
    return y


def kernel(x: np.ndarray) -> np.ndarray:
    from concourse.bass_utils import run_bass_kernel_spmd

    nc = build_bass()
    x = np.asarray(x, dtype=np.float32)
    res = run_bass_kernel_spmd(nc, _shard_inputs(x), list(range(NCORES)))
    return _gather_outputs(res.results)


# revision 20
# speedup vs baseline: 1.1636x; 1.0193x over previous
"""Derivative1D kernel for Trainium2 (8 NeuronCores, data-parallel over batch).

Reference: y = x[:, 1:, :] - x[:, :-1, :] with x of shape (64, 16384, 32) f32.

Key observation: flattening each batch's (L, C) block to a contiguous array,
y_flat[i] = x_flat[i + C] - x_flat[i].  The row the reference drops (l = L-1)
absorbs the batch-boundary garbage, so the whole per-core problem is one flat
shifted subtraction; the garbage rows are sliced off on the host.

Sharding: batch axis across 8 cores (8 batches per core, no communication).

Per core: chunks of the flat input are loaded with *overlapping* rows
(partition p reads F+C elements starting at p*F) so the shift-by-C stays
inside each partition; one vector subtract per chunk; contiguous store.

Raw Bass (no TileContext): the walrus codegen on this path rejects
instructions carrying more than one sync wait, so every wait is an explicit
standalone wait_ge.  Per-slot semaphores are used because DMA completions
are not FIFO across queues — each wait certifies exactly the DMA it needs.
"""

import numpy as np

B, L, C = 64, 16384, 32
NCORES = 8
BLOC = B // NCORES            # batches per core
N = BLOC * L * C              # flat elements per core
PAD = C                       # shift amount = channel count
P = 128                       # SBUF partitions
F = 8192                      # free elements per partition per chunk
CHUNK = P * F                 # elements per chunk
NCHUNKS = N // CHUNK
NBUF = 3                      # in-tile slots
NOBUF = 3                     # out-tile slots

assert N % CHUNK == 0

_built = None


def build_bass():
    global _built
    if _built is not None:
        return _built
    import concourse.bass as bass
    import concourse.mybir as mybir
    from contextlib import ExitStack

    f32 = mybir.dt.float32
    nc = bass.Bass()
    x = nc.declare_dram_parameter("x", [N + PAD], f32, isOutput=False)
    y = nc.declare_dram_parameter("y", [N], f32, isOutput=True)

    with ExitStack() as ctx:
        A = [
            ctx.enter_context(nc.sbuf_tensor(f"A{i}", [P, F + PAD], f32))
            for i in range(NBUF)
        ]
        Y = [
            ctx.enter_context(nc.sbuf_tensor(f"Y{i}", [P, F], f32))
            for i in range(NOBUF)
        ]
        LS = [ctx.enter_context(nc.semaphore(f"LS{i}")) for i in range(NBUF)]
        SS = [ctx.enter_context(nc.semaphore(f"SS{i}")) for i in range(NOBUF)]
        VS = ctx.enter_context(nc.semaphore("VS"))

        # no_gpsimd_drain: skip the expensive GpSimd dge_drain + full
        # EVSEM butterfly at block exit (no SWDGE DMAs are in flight;
        # HWDGE completion is certified by the explicit SS waits).
        block = ctx.enter_context(nc.Block(no_gpsimd_drain=True))

        @block.sync
        def _(sync):
            # Single HWDGE ring: the ring drains near-serially, so reads
            # and writes alternate at whole-DMA granularity (4 MiB) —
            # coarse enough that HBM R/W turnaround is negligible.  (A
            # dual-ring split was measured worse: both rings interleave
            # at packet granularity on every SDMA engine.)
            for k in range(min(NBUF, NCHUNKS)):
                sync.dma_start(
                    out=A[k][:],
                    in_=bass.AP(x, k * CHUNK, [[F, P], [1, F + PAD]]),
                ).then_inc(LS[k], 16)
            for k in range(NCHUNKS):
                # Store chunk k once its subtract is done; the wait also
                # certifies (in program order) that the next load's input
                # slot has been fully read by its subtract.
                sync.wait_ge(VS, k + 1)
                sync.dma_start(
                    out=bass.AP(y, k * CHUNK, [[F, P], [1, F]]),
                    in_=Y[k % NOBUF][:],
                ).then_inc(SS[k % NOBUF], 16)
                kn = k + NBUF
                if kn < NCHUNKS:
                    sync.dma_start(
                        out=A[kn % NBUF][:],
                        in_=bass.AP(x, kn * CHUNK, [[F, P], [1, F + PAD]]),
                    ).then_inc(LS[kn % NBUF], 16)
            # Epilogue: all stores complete before the kernel exits.
            for i in range(NOBUF):
                n_stores = len(range(i, NCHUNKS, NOBUF))
                sync.wait_ge(SS[i], 16 * n_stores)

        @block.vector
        def _(vector):
            for k in range(NCHUNKS):
                vector.wait_ge(LS[k % NBUF], 16 * (k // NBUF + 1))
                if k >= NOBUF:
                    # WAR: the out slot must have been drained by its store.
                    vector.wait_ge(SS[k % NOBUF], 16 * ((k - NOBUF) // NOBUF + 1))
                a = A[k % NBUF]
                vector.tensor_sub(
                    Y[k % NOBUF][:], a[:, PAD : F + PAD], a[:, 0:F]
                ).then_inc(VS, 1)

    _built = nc
    return nc


def _shard_inputs(x: np.ndarray) -> list[dict]:
    in_maps = []
    for c in range(NCORES):
        shard = np.empty(N + PAD, dtype=np.float32)
        shard[:N] = x[c * BLOC : (c + 1) * BLOC].reshape(-1)
        shard[N:] = 0.0
        in_maps.append({"x": shard})
    return in_maps


def _gather_outputs(results: list[dict]) -> np.ndarray:
    y = np.empty((B, L - 1, C), dtype=np.float32)
    for c in range(NCORES):
        y[c * BLOC : (c + 1) * BLOC] = (
            results[c]["y"].reshape(BLOC, L, C)[:, : L - 1, :]
        )
    return y


def kernel(x: np.ndarray) -> np.ndarray:
    from concourse.bass_utils import run_bass_kernel_spmd

    nc = build_bass()
    x = np.asarray(x, dtype=np.float32)
    res = run_bass_kernel_spmd(nc, _shard_inputs(x), list(range(NCORES)))
    return _gather_outputs(res.results)
